# revision 1
# baseline (speedup 1.0000x reference)
"""Multi-head self-attention (B=4, T=2048, D=1024, H=16) on 8 TRN2 NeuronCores.

Reference quirk: softmax normalizes over the QUERY axis (dim=2 of
[B,H,T1,T2]), i.e. attn[q,k] = exp(s[q,k]) / sum_q' exp(s[q',k]).

Sharding (fully SPMD, one NEFF for all 8 cores):
  core c -> batch b = c//2, head-group g = c%2 (8 heads = 512 cols of Wq/Wk/Wv).
  Host pre-slices AND pre-transposes per-core inputs (xT, wqT/wkT/wvT), runs
  the kernel, and stitches the 8 [T, 512] output shards back together.

Device algorithm per core (all matmuls float32r = FP22, full PE rate @ N>=512):
  1. QT/KT [128e, T] per head-pair (partition = head dims of 2 heads),
     V [128t, 512e] natural, via PE from xT / w*T tiles.
  2. Per head-pair, per 128-wide key chunk:
       S' = K @ Q^T chunk [128 k, T q] in PSUM (row-tiled pair: head A rows
       0-63, head B rows 64-127 of the PE array, concurrent),
       P = exp(SCALE * S') via ScalarE PSUM->SBUF, with accum_out giving the
       per-key row-sums (the softmax denominators Z[k]) for free,
       V'[k,:] = V[k,:] / Z[k]  (normalization folded into V: tiny),
       outT[d, q] += V'^T-style matmul: lhsT=V' [128k, 64d], rhs=P [128k, 512q]
       accumulated over all 16 chunks in PSUM (col-tiled head pair).
  3. Epilogue: outT -> PE transpose -> out natural [T, 512] -> DRAM.
"""

import numpy as np

B, T, D, H = 4, 2048, 1024, 16
DH = D // H
SCALE = 1.0 / (DH**0.5)
N_CORES = 8
E = D // 2  # 512 output cols per core (8 heads)
N_PAIRS = 4  # head-pairs per core
N_DC = D // 128  # 8 contraction chunks for projections
N_KC = T // 128  # 16 key chunks
QB = 1024  # exp free-dim block (2 PSUM banks)

_built = None  # (nc,) cache so repeat kernel() calls skip rebuild/recompile


def _np_reference(x, padding_mask, Wq, Wk, Wv):
    """Pure-numpy fallback, used only if the mask is not all-ones."""
    x64 = x.astype(np.float64)
    Q = (x64 @ Wq.T.astype(np.float64)).reshape(B, T, H, DH).transpose(0, 2, 1, 3)
    K = (x64 @ Wk.T.astype(np.float64)).reshape(B, T, H, DH).transpose(0, 2, 1, 3)
    V = (x64 @ Wv.T.astype(np.float64)).reshape(B, T, H, DH).transpose(0, 2, 1, 3)
    s = np.einsum("bhqd,bhkd->bhqk", Q, K) * SCALE
    s = np.where(padding_mask[:, None, :, :] == 0, -np.inf, s)
    s = s - s.max(axis=2, keepdims=True)
    p = np.exp(s)
    p = p / p.sum(axis=2, keepdims=True)
    out = np.einsum("bhqk,bhkd->bhqd", p, V)
    return out.transpose(0, 2, 1, 3).reshape(B, T, D).astype(np.float32)


def _split_multi_waits(nc):
    """Walrus caps sync waits at 1 per instruction; Tile's tail drain can carry
    several. Move the extras onto single-wait drains appended to the previous
    basic block (same engine, earlier in program order)."""
    import concourse.mybir as mybir

    blocks = list(nc.m.functions[0].blocks)
    for bi, blk in enumerate(blocks):
        for inst in blk.instructions:
            if type(inst).__name__ not in ("InstDrain", "InstNoOp", "InstEventSemaphore"):
                # compute/DMA instructions accept multiple waits; only the
                # CTRL-lowered ones (drain/nop/eventsem) are capped at 1-2.
                continue
            si = inst.sync_info
            if si is not None and si.on_wait and len(si.on_wait) > 1:
                waits = list(si.on_wait)
                keep, extra = waits[-1], waits[:-1]
                assert all(w.wait_mode == "sem-ge-imm" for w in extra), extra
                si.on_wait = [keep]
                assert bi > 0, "multi-wait in first block"
                prev = blocks[bi - 1]
                for j, w in enumerate(extra):
                    d = mybir.InstDrain(
                        name=f"{inst.name}-ws{j}",
                        engine=inst.engine,
                        sync_info=mybir.SyncInfo(on_wait=[w], on_update=[]),
                    )
                    prev.add_instruction(d)


def _build_kernel(tc, xT, wqT, wkT, wvT, out):
    import concourse.bass as bass  # noqa: F401
    import concourse.mybir as mybir
    from concourse.masks import make_identity

    nc = tc.nc
    FP = mybir.dt.float32
    FR = mybir.dt.float32r
    BF = mybir.dt.bfloat16
    Exp = mybir.ActivationFunctionType.Exp

    singles = tc.alloc_tile_pool(name="singles", bufs=1)
    ident = singles.tile([128, 128], FP, name="ident")
    make_identity(nc, ident)

    # long-lived pools
    xw = tc.alloc_tile_pool(name="xw", bufs=1)
    wp = tc.alloc_tile_pool(name="wp", bufs=3)
    qkv = tc.alloc_tile_pool(name="qkv", bufs=1)
    # PSUM: S pool (2x [128,QB] = 4 banks) shared by scores, projections and
    # epilogue transposes; acc pool (2x [128,QB] = 4 banks) for outT.
    sps = tc.alloc_tile_pool(name="sps", bufs=2, space="PSUM")
    accps = tc.alloc_tile_pool(name="accps", bufs=1, space="PSUM")
    pp = tc.alloc_tile_pool(name="pp", bufs=8)
    zp = tc.alloc_tile_pool(name="zp", bufs=4)
    vpp = tc.alloc_tile_pool(name="vpp", bufs=4)
    op = tc.alloc_tile_pool(name="op", bufs=2)

    # ---- loads ----
    xTs = []
    for dc in range(N_DC):
        t = xw.tile([128, T], FR, name=f"xT{dc}", tag=f"x{dc}")
        nc.sync.dma_start(out=t, in_=xT[dc * 128 : (dc + 1) * 128, :])
        xTs.append(t)

    def load_w(wap, label):
        ws = []
        for dc in range(N_DC):
            t = wp.tile([128, E], FR, name=f"{label}{dc}", tag=f"w{dc}")
            nc.sync.dma_start(out=t, in_=wap[dc * 128 : (dc + 1) * 128, :])
            ws.append(t)
        return ws

    wq = load_w(wqT, "wq")
    wk = load_w(wkT, "wk")
    wv = load_w(wvT, "wv")

    # ---- projection emitters (psum borrowed from the S pool tag) ----
    def project_eT_tile(ws, pair, tt, et):
        """One [128, 512] t-block of QT/KT pair tile `et` (bf16 [128, T])."""
        ps = sps.tile([128, QB], FP, name=f"ps_{et.tensor.name}_{tt}", tag="s")
        for dc in range(N_DC):
            nc.tensor.matmul(
                ps[:, 0:512],
                ws[dc][:, pair * 128 : (pair + 1) * 128],
                xTs[dc][:, tt * 512 : (tt + 1) * 512],
                start=(dc == 0),
                stop=(dc == N_DC - 1),
            )
        nc.vector.tensor_copy(et[:, tt * 512 : (tt + 1) * 512], ps[:, 0:512])

    def project_v_tile(tt):
        v = qkv.tile([128, E], BF, name=f"v{tt}", tag=f"v{tt}")
        ps = sps.tile([128, QB], FP, name=f"ps_v{tt}", tag="s")
        for dc in range(N_DC):
            nc.tensor.matmul(
                ps[:, 0:512],
                xTs[dc][:, tt * 128 : (tt + 1) * 128],
                wv[dc],
                start=(dc == 0),
                stop=(dc == N_DC - 1),
            )
        nc.vector.tensor_copy(v, ps[:, 0:512])
        return v

    QT = [None] * N_PAIRS
    KT = [None] * N_PAIRS
    V = [None] * N_KC

    def make_pair_qtkt(pair):
        QT[pair] = qkv.tile([128, T], BF, name=f"qt{pair}", tag=f"qt{pair}")
        KT[pair] = qkv.tile([128, T], BF, name=f"kt{pair}", tag=f"kt{pair}")
        for tt in range(4):
            project_eT_tile(wq, pair, tt, QT[pair])
        for tt in range(4):
            project_eT_tile(wk, pair, tt, KT[pair])

    # pair 0 QT/KT upfront, V[0] upfront; the rest interleaves into the
    # attention chunk stream below.
    make_pair_qtkt(0)
    V[0] = project_v_tile(0)

    for p in range(N_PAIRS):
        acc = [
            accps.tile([128, QB], FP, name=f"acc{qb}_{p}", tag=f"acc{qb}")
            for qb in range(2)
        ]
        for c in range(N_KC):
            zs = zp.tile([128, 4], FP, name=f"zs_{p}_{c}", tag="zs")
            ptiles = {}
            for qb in range(2):
                # Row-tiled concurrent pair: head A (PE rows 0-63) and head B
                # (rows 64-127) stream simultaneously into DIFFERENT psum
                # banks; alternating MMs let LDWEIGHTS hide under the other
                # row-group's matmul.
                sAB = [
                    sps.tile([128, QB], FP, name=f"s_{p}_{c}_{hi}_{qb}", tag="s")
                    for hi in range(2)
                ]
                for qt in range(2):
                    q0 = qb * QB + qt * 512
                    for hi, base in enumerate((0, 64)):
                        nc.tensor.matmul(
                            sAB[hi][:, qt * 512 : (qt + 1) * 512],
                            KT[p][base : base + 64, c * 128 : (c + 1) * 128],
                            QT[p][base : base + 64, q0 : q0 + 512],
                            start=True,
                            stop=True,
                            tile_position=(base, 0),
                        )
                for hi in range(2):
                    pt = pp.tile([128, QB], BF, name=f"p_{p}_{c}_{hi}_{qb}", tag="p")
                    nc.scalar.activation(
                        out=pt,
                        in_=sAB[hi],
                        func=Exp,
                        scale=SCALE,
                        accum_out=zs[:, 2 * hi + qb : 2 * hi + qb + 1],
                    )
                    ptiles[(hi, qb)] = pt
            # Z = qb0 + qb1 partial sums; r = 1/Z; V' = V * r (zero-padded)
            za = zp.tile([128, 2], FP, name=f"za_{p}_{c}", tag="za")
            nc.vector.tensor_add(za[:, 0:1], zs[:, 0:1], zs[:, 1:2])
            nc.vector.tensor_add(za[:, 1:2], zs[:, 2:3], zs[:, 3:4])
            rz = zp.tile([128, 2], FP, name=f"rz_{p}_{c}", tag="rz")
            nc.vector.reciprocal(out=rz, in_=za)
            vsrc = V[c]
            vpad = []
            for hi in range(2):
                vt = vpp.tile([128, 128], BF, name=f"vp{hi}_{p}_{c}", tag=f"vp{hi}")
                lo, hi_ = (0, 64) if hi == 0 else (64, 128)
                zlo, zhi = (64, 128) if hi == 0 else (0, 64)
                nc.gpsimd.memset(vt[:, zlo:zhi], 0.0)
                nc.vector.tensor_scalar_mul(
                    vt[:, lo:hi_],
                    vsrc[:, p * 128 + lo : p * 128 + hi_],
                    rz[:, hi : hi + 1],
                )
                vpad.append(vt)
            for qb in range(2):
                for qt in range(2):
                    for hi in range(2):
                        nc.tensor.matmul(
                            acc[qb][:, qt * 512 : (qt + 1) * 512],
                            vpad[hi],
                            ptiles[(hi, qb)][:, qt * 512 : (qt + 1) * 512],
                            start=(c == 0 and hi == 0),
                            stop=(c == N_KC - 1 and hi == 1),
                        )
            # ---- interleaved background work ----
            if p == 0 and c + 1 < N_KC:
                V[c + 1] = project_v_tile(c + 1)
            if p < N_PAIRS - 1 and c in (8, 10):
                # next pair's QT/KT, emitted mid-stream (2 bursts of 4 tiles)
                if c == 8:
                    QT[p + 1] = qkv.tile(
                        [128, T], BF, name=f"qt{p+1}", tag=f"qt{p+1}"
                    )
                    for tt in range(4):
                        project_eT_tile(wq, p + 1, tt, QT[p + 1])
                else:
                    KT[p + 1] = qkv.tile(
                        [128, T], BF, name=f"kt{p+1}", tag=f"kt{p+1}"
                    )
                    for tt in range(4):
                        project_eT_tile(wk, p + 1, tt, KT[p + 1])
        # epilogue: outT [128 (2h x 64d), T] -> transpose -> out natural
        ot = op.tile([128, T], FP, name=f"ot_{p}", tag="ot")
        nc.vector.tensor_copy(ot[:, 0:QB], acc[0])
        nc.vector.tensor_copy(ot[:, QB : 2 * QB], acc[1])
        for g in range(2):
            tps = sps.tile([128, QB], FP, name=f"tp_{p}_{g}", tag="s")
            for j in range(8):
                blk = g * 8 + j
                nc.tensor.transpose(
                    tps[:, j * 128 : (j + 1) * 128],
                    ot[:, blk * 128 : (blk + 1) * 128],
                    ident,
                )
            onb = op.tile([128, QB], FP, name=f"onb_{p}_{g}", tag="onb")
            nc.vector.tensor_copy(onb, tps)
            for j in range(8):
                blk = g * 8 + j
                nc.sync.dma_start(
                    out=out[blk * 128 : (blk + 1) * 128, p * 128 : (p + 1) * 128],
                    in_=onb[:, j * 128 : (j + 1) * 128],
                )

    for pool in (op, vpp, zp, pp, accps, sps, qkv, wp, xw, singles):
        pool.release()


def build():
    import concourse.bacc as bacc
    import concourse.mybir as mybir
    import concourse.tile as tile

    nc = bacc.Bacc("TRN2", target_bir_lowering=False, debug=False)
    FP = mybir.dt.float32
    FR = mybir.dt.float32r
    xT = nc.dram_tensor("xT", [D, T], FR, kind="ExternalInput").ap()
    wqT = nc.dram_tensor("wqT", [D, E], FR, kind="ExternalInput").ap()
    wkT = nc.dram_tensor("wkT", [D, E], FR, kind="ExternalInput").ap()
    wvT = nc.dram_tensor("wvT", [D, E], FR, kind="ExternalInput").ap()
    out = nc.dram_tensor("out", [T, E], FP, kind="ExternalOutput").ap()
    with tile.TileContext(nc) as tc:
        _build_kernel(tc, xT, wqT, wkT, wvT, out)
    nc.compile()
    _split_multi_waits(nc)
    return nc


def _get_nc():
    global _built
    if _built is None:
        _built = build()
    return _built


def make_in_maps(x, Wq, Wk, Wv):
    in_maps = []
    for c in range(N_CORES):
        b, g = divmod(c, 2)
        e0 = E * g
        in_maps.append(
            {
                "xT": np.ascontiguousarray(x[b].T),
                "wqT": np.ascontiguousarray(Wq[e0 : e0 + E, :].T),
                "wkT": np.ascontiguousarray(Wk[e0 : e0 + E, :].T),
                "wvT": np.ascontiguousarray(Wv[e0 : e0 + E, :].T),
            }
        )
    return in_maps


def assemble_out(results):
    out = np.empty((B, T, D), np.float32)
    for c in range(N_CORES):
        b, g = divmod(c, 2)
        e0 = E * g
        out[b][:, e0 : e0 + E] = results[c]["out"]
    return out


def kernel(x, padding_mask, Wq, Wk, Wv):
    x = np.asarray(x, dtype=np.float32)
    padding_mask = np.asarray(padding_mask, dtype=np.float32)
    Wq = np.asarray(Wq, dtype=np.float32)
    Wk = np.asarray(Wk, dtype=np.float32)
    Wv = np.asarray(Wv, dtype=np.float32)
    if not np.all(padding_mask == 1.0):
        return _np_reference(x, padding_mask, Wq, Wk, Wv)

    from concourse.bass_utils import run_bass_kernel_spmd

    nc = _get_nc()
    in_maps = make_in_maps(x, Wq, Wk, Wv)
    res = run_bass_kernel_spmd(nc, in_maps, list(range(N_CORES)))
    return assemble_out(res.results)



# revision 2
# speedup vs baseline: 1.3120x; 1.3120x over previous
"""Multi-head self-attention (B=4, T=2048, D=1024, H=16) on 8 TRN2 NeuronCores.

Reference quirk: softmax normalizes over the QUERY axis (dim=2 of
[B,H,T1,T2]), i.e. attn[q,k] = exp(s[q,k]) / sum_q' exp(s[q',k]).

Sharding (fully SPMD, one NEFF for all 8 cores):
  core c -> batch b = c//2, head-group g = c%2 (8 heads = 512 cols of Wq/Wk/Wv).
  Host pre-slices AND pre-transposes per-core inputs (xT, wqT/wkT/wvT), runs
  the kernel, and stitches the 8 transposed [512, T] output shards back
  together (host-side transpose: device emits outT, avoiding PE transposes).

Device algorithm per core (v2 — software-pipelined, dense-PE schedule):
  1. QT/KT [128e, T] per head-pair, V [128t, 512e] natural, via PE.
     Pair-0 QT/KT and V[0:4] up-front; remaining V tiles and the next
     pair's QT/KT are dribbled one [128,512] block per chunk into the
     attention stream to keep the PE busy (HAM stays un-throttled).
  2. Per head-pair, per 128-wide key chunk:
       S = K @ Q^T [128 k, 1024 q] per (head, q-half) in PSUM,
       P = exp(SCALE * S) via ScalarE PSUM->SBUF (accum_out -> Z row-sums),
       V'[k,:] = V[k,:] / Z[k] into persistent zero-padded vpad tiles,
       outT[d, q] += vpad^T @ P accumulated over 16 chunks in PSUM.
     Emission is pipelined: scores for chunk c+1 are issued between the
     exp and AV of chunk c so neither PE nor ACT queues behind the other.
  3. Epilogue per pair: acc -> SBUF copy -> DMA to outT rows (no transpose).
"""

import numpy as np

B, T, D, H = 4, 2048, 1024, 16
DH = D // H
SCALE = 1.0 / (DH**0.5)
N_CORES = 8
E = D // 2  # 512 output cols per core (8 heads)
N_PAIRS = 4  # head-pairs per core
N_DC = D // 128  # 8 contraction chunks for projections
N_KC = T // 128  # 16 key chunks
QB = 1024  # exp free-dim block (2 PSUM banks)
V_PRE = 4  # V tiles projected in the prologue; rest dribbled

_built = None  # (nc,) cache so repeat kernel() calls skip rebuild/recompile


def _np_reference(x, padding_mask, Wq, Wk, Wv):
    """Pure-numpy fallback, used only if the mask is not all-ones."""
    x64 = x.astype(np.float64)
    Q = (x64 @ Wq.T.astype(np.float64)).reshape(B, T, H, DH).transpose(0, 2, 1, 3)
    K = (x64 @ Wk.T.astype(np.float64)).reshape(B, T, H, DH).transpose(0, 2, 1, 3)
    V = (x64 @ Wv.T.astype(np.float64)).reshape(B, T, H, DH).transpose(0, 2, 1, 3)
    s = np.einsum("bhqd,bhkd->bhqk", Q, K) * SCALE
    s = np.where(padding_mask[:, None, :, :] == 0, -np.inf, s)
    s = s - s.max(axis=2, keepdims=True)
    p = np.exp(s)
    p = p / p.sum(axis=2, keepdims=True)
    out = np.einsum("bhqk,bhkd->bhqd", p, V)
    return out.transpose(0, 2, 1, 3).reshape(B, T, D).astype(np.float32)


def _split_multi_waits(nc):
    """Walrus caps sync waits at 1 per instruction; Tile's tail drain can carry
    several. Move the extras onto single-wait drains appended to the previous
    basic block (same engine, earlier in program order)."""
    import concourse.mybir as mybir

    blocks = list(nc.m.functions[0].blocks)
    for bi, blk in enumerate(blocks):
        for inst in blk.instructions:
            if type(inst).__name__ not in ("InstDrain", "InstNoOp", "InstEventSemaphore"):
                continue
            si = inst.sync_info
            if si is not None and si.on_wait and len(si.on_wait) > 1:
                waits = list(si.on_wait)
                keep, extra = waits[-1], waits[:-1]
                assert all(w.wait_mode == "sem-ge-imm" for w in extra), extra
                si.on_wait = [keep]
                assert bi > 0, "multi-wait in first block"
                prev = blocks[bi - 1]
                for j, w in enumerate(extra):
                    d = mybir.InstDrain(
                        name=f"{inst.name}-ws{j}",
                        engine=inst.engine,
                        sync_info=mybir.SyncInfo(on_wait=[w], on_update=[]),
                    )
                    prev.add_instruction(d)


def _build_kernel(tc, xT, wqT, wkT, wvT, outT):
    import concourse.bass as bass  # noqa: F401
    import concourse.mybir as mybir

    nc = tc.nc
    FP = mybir.dt.float32
    FR = mybir.dt.float32r
    BF = mybir.dt.bfloat16
    Exp = mybir.ActivationFunctionType.Exp

    # long-lived pools
    singles = tc.alloc_tile_pool(name="singles", bufs=1)
    xw = tc.alloc_tile_pool(name="xw", bufs=1)
    wp = tc.alloc_tile_pool(name="wp", bufs=3)
    qkv = tc.alloc_tile_pool(name="qkv", bufs=1)
    sps = tc.alloc_tile_pool(name="sps", bufs=2, space="PSUM")
    accps = tc.alloc_tile_pool(name="accps", bufs=1, space="PSUM")
    pp = tc.alloc_tile_pool(name="pp", bufs=8)
    zp = tc.alloc_tile_pool(name="zp", bufs=4)
    op = tc.alloc_tile_pool(name="op", bufs=2)

    # ---- loads (xT first: it is the long pole every projection gates on) ----
    xTs = []
    for dc in range(N_DC):
        t = xw.tile([128, T], FR, name=f"xT{dc}", tag=f"x{dc}")
        nc.sync.dma_start(out=t, in_=xT[dc * 128 : (dc + 1) * 128, :])
        xTs.append(t)

    def load_w(wap, label):
        ws = []
        for dc in range(N_DC):
            t = wp.tile([128, E], FR, name=f"{label}{dc}", tag=f"w{dc}")
            nc.sync.dma_start(out=t, in_=wap[dc * 128 : (dc + 1) * 128, :])
            ws.append(t)
        return ws

    wq = load_w(wqT, "wq")
    wk = load_w(wkT, "wk")
    wv = load_w(wvT, "wv")

    # persistent zero-padded V' tiles: [parity][hi], data half written per chunk
    vpads = [[None, None], [None, None]]
    for par in range(2):
        for hi in range(2):
            vt = singles.tile([128, 128], BF, name=f"vp{par}{hi}")
            nc.gpsimd.memset(vt, 0.0)
            vpads[par][hi] = vt

    # ---- projection emitters (psum borrowed from the S pool tag) ----
    def project_eT_tile(ws, pair, tt, et):
        """One [128, 512] t-block of QT/KT pair tile `et` (bf16 [128, T])."""
        ps = sps.tile([128, QB], FP, name=f"ps_{et.tensor.name}_{tt}", tag="s")
        for dc in range(N_DC):
            nc.tensor.matmul(
                ps[:, 0:512],
                ws[dc][:, pair * 128 : (pair + 1) * 128],
                xTs[dc][:, tt * 512 : (tt + 1) * 512],
                start=(dc == 0),
                stop=(dc == N_DC - 1),
            )
        nc.vector.tensor_copy(et[:, tt * 512 : (tt + 1) * 512], ps[:, 0:512])

    def project_v_tile(tt):
        v = qkv.tile([128, E], BF, name=f"v{tt}", tag=f"v{tt}")
        ps = sps.tile([128, QB], FP, name=f"ps_v{tt}", tag="s")
        for dc in range(N_DC):
            nc.tensor.matmul(
                ps[:, 0:512],
                xTs[dc][:, tt * 128 : (tt + 1) * 128],
                wv[dc],
                start=(dc == 0),
                stop=(dc == N_DC - 1),
            )
        nc.vector.tensor_copy(v, ps[:, 0:512])
        return v

    QT = [None] * N_PAIRS
    KT = [None] * N_PAIRS
    V = [None] * N_KC

    def alloc_pair(p):
        QT[p] = qkv.tile([128, T], BF, name=f"qt{p}", tag=f"qt{p}")
        KT[p] = qkv.tile([128, T], BF, name=f"kt{p}", tag=f"kt{p}")

    # prologue: pair-0 QT/KT + first V tiles (dense PE burst, warms the clock)
    alloc_pair(0)
    for tt in range(4):
        project_eT_tile(wq, 0, tt, QT[0])
    for tt in range(4):
        project_eT_tile(wk, 0, tt, KT[0])
    for tt in range(V_PRE):
        V[tt] = project_v_tile(tt)

    # dribble schedule: work[(p, c)] -> list of zero-arg emitters
    work = {}

    def add_work(p, c, fn):
        work.setdefault((p, c), []).append(fn)

    for c in range(N_KC - V_PRE):  # V[V_PRE..15] during pair 0, just-in-time
        add_work(0, c, (lambda tt: (lambda: V.__setitem__(tt, project_v_tile(tt))))(V_PRE + c))
    for p in range(N_PAIRS - 1):  # next pair QT/KT: 8 blocks at c=4..11
        add_work(p, 3, (lambda q: (lambda: alloc_pair(q)))(p + 1))
        for tt in range(4):
            add_work(p, 4 + tt, (lambda q, t2: (lambda: project_eT_tile(wq, q, t2, QT[q])))(p + 1, tt))
        for tt in range(4):
            add_work(p, 8 + tt, (lambda q, t2: (lambda: project_eT_tile(wk, q, t2, KT[q])))(p + 1, tt))

    # ---- attention emitters ----
    def scores_half(p, c, hi):
        """S tiles for one head of the pair: 2x [128k, 1024q] psum."""
        base = hi * 64
        out = {}
        for qb in range(2):
            s = sps.tile([128, QB], FP, name=f"s_{p}_{c}_{hi}_{qb}", tag="s")
            for qt in range(2):
                q0 = qb * QB + qt * 512
                nc.tensor.matmul(
                    s[:, qt * 512 : (qt + 1) * 512],
                    KT[p][base : base + 64, c * 128 : (c + 1) * 128],
                    QT[p][base : base + 64, q0 : q0 + 512],
                    start=True,
                    stop=True,
                    tile_position=(base, 0),
                )
            out[qb] = s
        return out

    def exps(p, c, stiles):
        zs = zp.tile([128, 4], FP, name=f"zs_{p}_{c}", tag="zs")
        ptiles = {}
        for hi in range(2):
            for qb in range(2):
                pt = pp.tile([128, QB], BF, name=f"p_{p}_{c}_{hi}_{qb}", tag="p")
                nc.scalar.activation(
                    out=pt,
                    in_=stiles[(hi, qb)],
                    func=Exp,
                    scale=SCALE,
                    accum_out=zs[:, 2 * hi + qb : 2 * hi + qb + 1],
                )
                ptiles[(hi, qb)] = pt
        return zs, ptiles

    def zchain(p, c, zs):
        za = zp.tile([128, 2], FP, name=f"za_{p}_{c}", tag="za")
        nc.vector.tensor_add(za[:, 0:1], zs[:, 0:1], zs[:, 1:2])
        nc.vector.tensor_add(za[:, 1:2], zs[:, 2:3], zs[:, 3:4])
        rz = zp.tile([128, 2], FP, name=f"rz_{p}_{c}", tag="rz")
        nc.vector.reciprocal(out=rz, in_=za)
        vt = vpads[c % 2]
        for hi in range(2):
            lo = hi * 64
            nc.vector.tensor_scalar_mul(
                vt[hi][:, lo : lo + 64],
                V[c][:, p * 128 + lo : p * 128 + lo + 64],
                rz[:, hi : hi + 1],
            )
        return vt

    def av_half(p, c, acc, vt, ptiles, hi):
        for qb in range(2):
            for qt in range(2):
                nc.tensor.matmul(
                    acc[qb][:, qt * 512 : (qt + 1) * 512],
                    vt[hi],
                    ptiles[(hi, qb)][:, qt * 512 : (qt + 1) * 512],
                    start=(c == 0 and hi == 0),
                    stop=(c == N_KC - 1 and hi == 1),
                )

    # ---- pipelined main loop ----
    stiles = {}
    for hi in range(2):
        for qb, s in scores_half(0, 0, hi).items():
            stiles[(hi, qb)] = s

    for p in range(N_PAIRS):
        acc = [
            accps.tile([128, QB], FP, name=f"acc{qb}_{p}", tag=f"acc{qb}")
            for qb in range(2)
        ]
        for c in range(N_KC):
            zs, ptiles = exps(p, c, stiles)
            nxt = (p, c + 1) if c + 1 < N_KC else (p + 1, 0)
            vt = zchain(p, c, zs)
            stiles = {}
            if nxt[0] < N_PAIRS:
                for qb, s in scores_half(*nxt, 0).items():
                    stiles[(0, qb)] = s
            av_half(p, c, acc, vt, ptiles, 0)
            if nxt[0] < N_PAIRS:
                for qb, s in scores_half(*nxt, 1).items():
                    stiles[(1, qb)] = s
            av_half(p, c, acc, vt, ptiles, 1)
            for fn in work.get((p, c), []):
                fn()
        # epilogue: outT rows for this pair (host transposes back)
        ot = op.tile([128, T], FP, name=f"ot_{p}", tag="ot")
        nc.vector.tensor_copy(ot[:, 0:QB], acc[0])
        nc.vector.tensor_copy(ot[:, QB : 2 * QB], acc[1])
        nc.sync.dma_start(out=outT[p * 128 : (p + 1) * 128, :], in_=ot)

    for pool in (op, zp, pp, accps, sps, qkv, wp, xw, singles):
        pool.release()


def build():
    import concourse.bacc as bacc
    import concourse.mybir as mybir
    import concourse.tile as tile

    nc = bacc.Bacc("TRN2", target_bir_lowering=False, debug=False)
    FP = mybir.dt.float32
    FR = mybir.dt.float32r
    xT = nc.dram_tensor("xT", [D, T], FR, kind="ExternalInput").ap()
    wqT = nc.dram_tensor("wqT", [D, E], FR, kind="ExternalInput").ap()
    wkT = nc.dram_tensor("wkT", [D, E], FR, kind="ExternalInput").ap()
    wvT = nc.dram_tensor("wvT", [D, E], FR, kind="ExternalInput").ap()
    outT = nc.dram_tensor("outT", [E, T], FP, kind="ExternalOutput").ap()
    with tile.TileContext(nc) as tc:
        _build_kernel(tc, xT, wqT, wkT, wvT, outT)
    nc.compile()
    _split_multi_waits(nc)
    return nc


def _get_nc():
    global _built
    if _built is None:
        _built = build()
    return _built


def make_in_maps(x, Wq, Wk, Wv):
    in_maps = []
    for c in range(N_CORES):
        b, g = divmod(c, 2)
        e0 = E * g
        in_maps.append(
            {
                "xT": np.ascontiguousarray(x[b].T),
                "wqT": np.ascontiguousarray(Wq[e0 : e0 + E, :].T),
                "wkT": np.ascontiguousarray(Wk[e0 : e0 + E, :].T),
                "wvT": np.ascontiguousarray(Wv[e0 : e0 + E, :].T),
            }
        )
    return in_maps


def assemble_out(results):
    out = np.empty((B, T, D), np.float32)
    for c in range(N_CORES):
        b, g = divmod(c, 2)
        e0 = E * g
        out[b][:, e0 : e0 + E] = results[c]["outT"].T
    return out


def kernel(x, padding_mask, Wq, Wk, Wv):
    x = np.asarray(x, dtype=np.float32)
    padding_mask = np.asarray(padding_mask, dtype=np.float32)
    Wq = np.asarray(Wq, dtype=np.float32)
    Wk = np.asarray(Wk, dtype=np.float32)
    Wv = np.asarray(Wv, dtype=np.float32)
    if not np.all(padding_mask == 1.0):
        return _np_reference(x, padding_mask, Wq, Wk, Wv)

    from concourse.bass_utils import run_bass_kernel_spmd

    nc = _get_nc()
    in_maps = make_in_maps(x, Wq, Wk, Wv)
    res = run_bass_kernel_spmd(nc, in_maps, list(range(N_CORES)))
    return assemble_out(res.results)
